# revision 64
# baseline (speedup 1.0000x reference)
"""Trainium2 Bass kernel: multi-head attention forward (B=2, S=2048, D=1024, H=16).

Sharding: 8 cores = data-parallel over batch (2) x tensor-parallel over heads
(4 head-groups of 4 heads).  Each core computes, for its batch b and head
group g: q/k/v projections for its 4 heads (column-sharded Wq/Wk/Wv),
causal-softmax attention for those heads, and a partial output projection
(row-sharded Wo).  The 4 partial outputs per batch are summed and the bias
added on the host.

All operands are bf16 (halves DMA vs f32, avoids the f32r small-tile matmul
penalty; total error ~1e-3, well under the 2e-2 gate).  The kernel is ONE
fused phase: the PE instruction stream weaves projection chunks, attention
steps, and output-projection tiles so the PE (bottleneck engine) stays busy
while the Activation engine works through the softmax exps.

Attention runs on head PAIRS over query blocks of W=512:
  scores for both heads of a pair land in one [128, 2*W] PSUM tile ->
  a single Exp instruction per sk-step covers both heads (halves the ACT
  per-instruction overhead).  v is augmented with 64 ones-columns so the
  PV matmul leaves the softmax denominator replicated in PSUM rows 64:127;
  the normalization is then a straight DVE reciprocal + multiply on 64
  partitions (no gpsimd partition_broadcast needed).
"""

import sys

sys.path.insert(0, "/opt/trn_rl_repo")

from collections import deque

import numpy as np

B, S, D = 2, 2048, 1024
H = 16
DH = 64
HL = 4  # heads per core
NCORES = 8

_PROGRAM_CACHE = {}


def build_program(S=S, D=D, HL=HL, DH=DH, debug_dumps=()):
    import concourse.tile as tile
    from concourse import bacc, mybir

    f32 = mybir.dt.float32
    bf16 = mybir.dt.bfloat16
    A = mybir.ActivationFunctionType
    Alu = mybir.AluOpType

    KD = D // 128          # contraction chunks for the projections
    M = HL * DH            # per-core projected width (256)
    MQ = M // 128          # qT/kT partition tiles (2)
    ST = S // 128          # 128-row s tiles
    W = min(512, S)        # query-block width
    NJ = S // W            # query blocks
    TPB = W // 128         # sk tiles per query block (4)
    VW = 128               # per-head v columns: DH data + 64 ones
    NPAIR = HL // 2
    PVLAG = 8              # sk-steps between scores and the matching PV
    scale = 1.0 / float(np.sqrt(DH))

    nc = bacc.Bacc("TRN2", target_bir_lowering=False, debug=False)
    xT = nc.dram_tensor("xT", (D, S), bf16, kind="ExternalInput").ap()
    wq = nc.dram_tensor("wq", (D, M), bf16, kind="ExternalInput").ap()
    wk = nc.dram_tensor("wk", (D, M), bf16, kind="ExternalInput").ap()
    wv = nc.dram_tensor("wv", (D, M), bf16, kind="ExternalInput").ap()
    wo = nc.dram_tensor("wo", (M, D), bf16, kind="ExternalInput").ap()
    out = nc.dram_tensor("out", (S, D), bf16, kind="ExternalOutput").ap()

    with tile.TileContext(nc) as tc:
        with (
            tc.tile_pool(name="persist", bufs=1) as mpool,
            tc.tile_pool(name="attn", bufs=10) as apool,
            tc.tile_pool(name="norm", bufs=2) as npool,
            tc.tile_pool(name="ostage", bufs=6) as opool,
            tc.tile_pool(name="proj_ps", bufs=2, space="PSUM") as ppool,
            tc.tile_pool(name="sc_ps", bufs=2, space="PSUM") as spool,
            tc.tile_pool(name="ctx_ps", bufs=1, space="PSUM") as cpool,
        ):
            xt = mpool.tile([128, KD, S], bf16, tag="xt")
            wq_sb = mpool.tile([128, KD, M], bf16, tag="wq")
            wk_sb = mpool.tile([128, KD, M], bf16, tag="wk")
            wv_sb = mpool.tile([128, KD, M], bf16, tag="wv")
            wo_sb = mpool.tile([128, MQ, D], bf16, tag="wo")
            qT_sb = mpool.tile([128, MQ, S], bf16, tag="qT")
            kT_sb = mpool.tile([128, MQ, S], bf16, tag="kT")
            v_sb = mpool.tile([128, ST, HL * VW], bf16, tag="v")
            ctx_sb = mpool.tile([128, MQ, S], bf16, tag="ctx")

            # ones columns for the PV denominator trick: PSUM rows 0:63 of
            # each ctx tile become the softmax denominator replicated 64x.
            # Ones FIRST: the custom-ISA reciprocal must read base partition
            # 0 (a base-64 input AP returns NaN on hardware).
            ones_ap = v_sb.rearrange("p st (h c) -> p (st h) c", c=VW)[:, :, 0:DH]
            nc.gpsimd.memset(ones_ap, 1.0)

            # ---------------- DMA issue (SP queue order = priority) --------
            # est_dma_x[n]: modeled completion time of x s-chunk n, used to
            # keep not-yet-landed filler units out of the in-order PE stream
            NX = S // 512  # x s-chunks
            wq_r = wq.rearrange("(k p) m -> p k m", p=128)
            wk_r = wk.rearrange("(k p) m -> p k m", p=128)
            wv_r = wv.rearrange("(k p) m -> p k m", p=128)
            xT_r = xT.rearrange("(k p) s -> p k s", p=128)
            t_dma = [300.0]  # issue latency
            est_dma_x = {}

            def dma_in(dst, src, n_desc, elem_bytes):
                # serial pipeline rate: max(seq 565, hwdge 625, transfer)
                mult = 2.0 if elem_bytes < 512 else 1.0
                t_dma[0] += max(625.0, n_desc / 16 * elem_bytes * mult / 22.5)
                nc.sync.dma_start(dst, src)

            XH = min(1024, S)  # x column-halves (big DMAs: HWDGE is serial)
            # first wq/wk chunks + a small first x chunk: the PE starts early
            # and the interleaved q/k prologue units consume each x chunk
            KH = KD // 2
            dma_in(wq_sb[:, 0], wq_r[:, 0], 128, 512)
            dma_in(wk_sb[:, 0], wk_r[:, 0], 128, 512)
            dma_in(xt[:, 0, 0:512], xT_r[:, 0, 0:512], 128, 1024)
            dma_in(wq_sb[:, 1:KH], wq_r[:, 1:KH], 128 * (KH - 1), 512)
            dma_in(wk_sb[:, 1:KH], wk_r[:, 1:KH], 128 * (KH - 1), 512)
            if XH > 512:
                dma_in(xt[:, 0, 512:XH], xT_r[:, 0, 512:XH], 128,
                       2 * (XH - 512))
            for k in range(1, KD):
                if k == KH:
                    dma_in(wq_sb[:, KH:KD], wq_r[:, KH:KD], 128 * KH, 512)
                    dma_in(wk_sb[:, KH:KD], wk_r[:, KH:KD], 128 * KH, 512)
                dma_in(xt[:, k, 0:XH], xT_r[:, k, 0:XH], 128, 2 * XH)
            for n in range(XH // 512):
                est_dma_x[n] = t_dma[0]
            dma_in(wv_sb[:], wv_r, 128 * KD, 512)
            dma_in(wo_sb[:], wo.rearrange("(k p) d -> p k d", p=128), 256, 2048)
            if S > XH:
                for k in range(KD):
                    dma_in(xt[:, k, XH:S], xT_r[:, k, XH:S], 128, 2 * (S - XH))
                for n in range(XH // 512, NX):
                    est_dma_x[n] = t_dma[0]

            # ---------------- work units (emitted once, on demand) --------
            # j0/j1 first (their x columns land in the first DMA half); the
            # ACT-heaviest block (j3) mid-schedule where filler supply exists;
            # j2 last so the final block's ACT surplus (= serial tail) is small
            j_order = list(range(NJ))
            if NJ >= 4:
                j_order = j_order[:-2] + [j_order[-1], j_order[-2]]
            emitted = set()
            norms_done = set()
            o_defer = []
            osb_tiles = {}
            osb_done = {}
            ocopy_rot = [0]

            def emit_qk_unit(which, m, n):
                w_sb, dst = (wq_sb, qT_sb) if which == "q" else (wk_sb, kT_sb)
                ps = ppool.tile([128, 512], f32, tag="p", name="ps_qk")
                sl = slice(n * 512, n * 512 + 512)
                for k in range(KD):
                    nc.tensor.matmul(
                        ps[:],
                        w_sb[:, k, m * 128:(m + 1) * 128],
                        xt[:, k, sl],
                        start=(k == 0),
                        stop=(k == KD - 1),
                    )
                nc.vector.tensor_copy(dst[:, m, sl], ps[:])

            def emit_v_unit(st):
                ps = ppool.tile([128, 512], f32, tag="p", name="ps_v")
                for k in range(KD):
                    nc.tensor.matmul(
                        ps[:, 0:M],
                        xt[:, k, st * 128:(st + 1) * 128],
                        wv_sb[:, k, :],
                        start=(k == 0),
                        stop=(k == KD - 1),
                    )
                vdst = v_sb[:, st].rearrange("p (h c) -> p h c", h=HL)[:, :, DH:VW]
                nc.vector.tensor_copy(
                    vdst, ps[:, 0:M].rearrange("p (h c) -> p h c", h=HL)
                )

            def get_osb(st):
                if st not in osb_tiles:
                    osb_tiles[st] = opool.tile([128, D], bf16, tag="o",
                                               name="o_sb")
                    osb_done[st] = set()
                return osb_tiles[st]

            def stage_copy(dst, ps, allow_act):
                # mid-schedule: DVE only (Pool copies would head-of-line
                # block the causal masks that gate PVs).  In the tail
                # (allow_act: final j, exps+masks done) rotate across
                # DVE/Pool/ACT — a pure-DVE chain at 658ns/unit is slower
                # than the PE's 427ns and would serialize the tail.
                if not allow_act:
                    nc.vector.tensor_copy(dst, ps[:])
                    return
                eng = ocopy_rot[0] % 2
                ocopy_rot[0] += 1
                if eng == 1:
                    nc.scalar.activation(dst, ps[:], A.Copy)
                else:
                    nc.vector.tensor_copy(dst, ps[:])

            def maybe_store(st):
                if len(osb_done[st]) == D // 512:
                    nc.sync.dma_start(
                        out[st * 128:(st + 1) * 128, :], osb_tiles[st][:]
                    )

            def emit_o_unit(st, n):
                ps = ppool.tile([128, 512], f32, tag="p", name="ps_o")
                for p2 in range(MQ):
                    nc.tensor.matmul(
                        ps[:],
                        ctx_sb[:, p2, st * 128:(st + 1) * 128],
                        wo_sb[:, p2, n * 512:(n + 1) * 512],
                        start=(p2 == 0),
                        stop=(p2 == MQ - 1),
                    )
                dst = get_osb(st)[:, n * 512:(n + 1) * 512]
                stage_copy(dst, ps, allow_act=(st // TPB == last_block_j))
                osb_done[st].add(n)
                if st // TPB == last_block_j:
                    # final block: store halves as they finish so the last
                    # DMA's pipeline latency starts as early as possible
                    nc.sync.dma_start(
                        out[st * 128:(st + 1) * 128, n * 512:(n + 1) * 512],
                        dst,
                    )
                else:
                    maybe_store(st)

            def emit_oa_unit(st, n):
                # last-j split: pair-0's contribution only (runs as filler
                # during the final block, before pair-1's norm exists)
                ps = ppool.tile([128, 512], f32, tag="p", name="ps_oa")
                nc.tensor.matmul(
                    ps[:], ctx_sb[:, 0, st * 128:(st + 1) * 128],
                    wo_sb[:, 0, n * 512:(n + 1) * 512], start=True, stop=True,
                )
                nc.vector.tensor_copy(get_osb(st)[:, n * 512:(n + 1) * 512],
                                      ps[:])

            def emit_ob_unit(st, n):
                ps = ppool.tile([128, 512], f32, tag="p", name="ps_ob")
                nc.tensor.matmul(
                    ps[:], ctx_sb[:, 1, st * 128:(st + 1) * 128],
                    wo_sb[:, 1, n * 512:(n + 1) * 512], start=True, stop=True,
                )
                dst = get_osb(st)[:, n * 512:(n + 1) * 512]
                nc.vector.tensor_tensor(dst, dst, ps[:], Alu.add)
                osb_done[st].add(n)
                maybe_store(st)

            UNIT_COST = {"q": 4096, "k": 4096, "v": 2048, "o": 1024,
                         "oa": 512, "ob": 512}
            est_pe = [1000.0]  # modeled PE progress (ns)

            def unit_chunk(key):
                """x s-chunk this unit depends on (None = no x dependency)."""
                if key[0] in ("q", "k"):
                    return key[2]
                if key[0] == "v":
                    return key[1] * 128 // 512
                return None

            def emit_unit(key):
                if key in emitted:
                    return 0
                emitted.add(key)
                if key[0] in ("q", "k"):
                    emit_qk_unit(key[0], key[1], key[2])
                elif key[0] == "v":
                    emit_v_unit(key[1])
                elif key[0] == "o":
                    emit_o_unit(key[1], key[2])
                elif key[0] == "oa":
                    emit_oa_unit(key[1], key[2])
                else:
                    emit_ob_unit(key[1], key[2])
                est_pe[0] += UNIT_COST[key[0]] * 0.417
                return UNIT_COST[key[0]]

            filler_q = deque()
            deferred = []  # (steps_left, fn)
            act_cum = [0.0]     # modeled ACT completion time of last exp
            exp_done = deque(maxlen=4)  # recent exp completion estimates

            def tick_deferred():
                for item in deferred[:]:
                    item[0] -= 1
                    if item[0] <= 0:
                        deferred.remove(item)
                        item[1]()

            def fill_until(target):
                """Pop eligible fillers until modeled PE progress >= target.

                Skips units whose x-chunk hasn't (per the DMA model) landed:
                a premature unit would head-of-line-block the in-order PE
                stream on its DMA.
                """
                scan = 0
                while filler_q and est_pe[0] < target and scan < len(filler_q):
                    key = filler_q[scan]
                    if key in emitted:
                        del filler_q[scan]
                        continue
                    ch = unit_chunk(key)
                    if ch is not None and est_dma_x.get(ch, 0.0) > est_pe[0]:
                        scan += 1
                        continue
                    del filler_q[scan]
                    emit_unit(key)

            def flush_fillers():
                while filler_q:
                    key = filler_q.popleft()
                    emit_unit(key)

            # ---------------- attention block -----------------------------
            def attention_block(g, j):
                for u in o_defer:
                    filler_q.append(u)
                o_defer.clear()
                h0 = 2 * g
                nski = TPB * (j + 1)
                ctxps = [
                    cpool.tile([128, W], f32, tag=f"c{i}", name="ctx_ps")
                    for i in range(2)
                ]
                pending = []

                def emit_pv(item):
                    ski, attn_t, ex0 = item
                    emit_unit(("v", ski))
                    for i in range(2):
                        nc.tensor.matmul(
                            ctxps[i][:, ex0:W],
                            v_sb[:, ski, (h0 + i) * VW:(h0 + i + 1) * VW],
                            attn_t[:, i, ex0:W],
                            start=(ski == 0),
                            stop=(ski == nski - 1),
                        )

                edone_blk = {}
                for ski in range(nski):
                    diag = ski >= TPB * j
                    ex0 = 128 * ski - j * W if diag else 0
                    wd = W - ex0
                    # the sc ring has 2 slots: the PE's scores for this step
                    # would stall until exp(step-2) retires.  Pre-pad the PE
                    # stream with filler so it reaches this point no earlier.
                    if len(exp_done) >= 2:
                        tgt = exp_done[-2] - 2 * wd * 0.417
                        fill_until(tgt)
                        est_pe[0] = max(est_pe[0], tgt)
                    sc = spool.tile([128, 2, W], f32, tag="s", name="sc_ps")
                    for i in range(2):
                        hrow = slice(64 * ((h0 + i) % 2), 64 * ((h0 + i) % 2) + 64)
                        nc.tensor.matmul(
                            sc[:, i, ex0:W],
                            kT_sb[hrow, g, ski * 128:(ski + 1) * 128],
                            qT_sb[hrow, g, j * W + ex0:(j + 1) * W],
                            start=True,
                            stop=True,
                        )
                    attn_t = apool.tile([128, 2, W], bf16, tag="a", name="attn_t")
                    nc.scalar.activation(
                        attn_t[:, :, ex0:W], sc[:, :, ex0:W], A.Exp, scale=scale
                    )
                    if diag:
                        cross_end = min(W, ex0 + 128)
                        for i in range(2):
                            nc.gpsimd.affine_select(
                                out=attn_t[:, i, ex0:cross_end],
                                in_=attn_t[:, i, ex0:cross_end],
                                compare_op=Alu.is_ge,
                                fill=0.0,
                                base=j * W + ex0 - 128 * ski,
                                pattern=[[1, cross_end - ex0]],
                                channel_multiplier=-1,
                            )
                    pending.append((ski, attn_t, ex0))
                    if len(pending) > PVLAG:
                        emit_pv(pending.pop(0))
                    # model this step: scores then exp (ACT runs behind PE)
                    est_pe[0] += 2 * wd * 0.417  # scores
                    e_done = max(act_cum[0], est_pe[0] + 150) + 2 * wd * 0.833 + 420
                    act_cum[0] = e_done
                    exp_done.append(e_done)
                    edone_blk[ski] = e_done
                    est_pe[0] += 2 * wd * 0.417  # PV
                    tick_deferred()
                for item in pending:
                    # the tail PVs wait on their exp+mask: pad the PE stream
                    # so it arrives no earlier (keeps the PE warm and busy)
                    tgt = edone_blk[item[0]] + 300
                    fill_until(tgt)
                    est_pe[0] = max(est_pe[0], tgt)
                    emit_pv(item)
                    est_pe[0] += 2 * (W - item[2]) * 0.417

                def norm_pool(g=g, j=j, ctxps=ctxps):
                    # both heads' replicated denominator rows leave PSUM into
                    # ONE stacked tile: a single ISA reciprocal covers both.
                    # (The ISA op needs a base-partition-0 SBUF input; for the
                    # final block ACT does the copies - it is idle by then -
                    # which shortens the serial DVE tail.)
                    cu = npool.tile([128, W], f32, tag="cu", name="cu")
                    for i in range(2):
                        if j == last_block_j:
                            nc.scalar.activation(cu[64 * i:64 * i + DH, :],
                                                 ctxps[i][0:DH, :], A.Copy)
                        else:
                            nc.vector.tensor_copy(cu[64 * i:64 * i + DH, :],
                                                  ctxps[i][0:DH, :])
                    norm_pool.cu = cu

                def norm_dve(g=g, j=j, ctxps=ctxps):
                    rcp = npool.tile([128, W], f32, tag="rcp", name="rcp")
                    nc.vector.reciprocal_approx_fast(
                        out=rcp[:], in_=norm_pool.cu[:]
                    )
                    for i in range(2):
                        po = 64 * ((h0 + i) % 2)
                        # in0 stays in PSUM: mixing base partitions is only
                        # disallowed when BOTH inputs are in SBUF
                        nc.vector.tensor_mul(
                            ctx_sb[po:po + DH, g, j * W:(j + 1) * W],
                            ctxps[i][DH:128, :],
                            rcp[64 * i:64 * i + DH, :],
                        )
                    # output projection for block j unlocks once BOTH pairs'
                    # norms for j are done (block order interleaves pairs)
                    norms_done.add((g, j))
                    if all((gg, j) in norms_done for gg in range(NPAIR)):
                        units = [("o", st, n)
                                 for st in range(j * TPB, (j + 1) * TPB)
                                 for n in range(D // 512)]
                        filler_q.extend(units[:len(units) - 2])
                        o_defer.extend(units[len(units) - 2:])

                deferred.append([1, norm_pool])
                deferred.append([3, norm_dve])

            # ---------------- the fused schedule ---------------------------
            # prologue: the first four q/k units with their contraction loops
            # interleaved, so each arriving x chunk feeds four matmuls while
            # the PE is otherwise DMA-starved.  Two of the accumulators
            # borrow scores-pool banks (attention hasn't started yet).
            pro = []
            for n in range(min(2, NX)):
                sl = slice(n * 512, n * 512 + 512)
                pq = (ppool if n == 0 else spool).tile(
                    [128, 512], f32, tag="p" if n == 0 else "s", name="pro_q")
                pk = (ppool if n == 0 else spool).tile(
                    [128, 512], f32, tag="p" if n == 0 else "s", name="pro_k")
                pro.append((n, sl, pq, pk))
                emitted.add(("q", 0, n))
                emitted.add(("k", 0, n))
            for k in range(KD):
                for n, sl, pq, pk in pro:
                    for ps, w_sb in ((pq, wq_sb), (pk, wk_sb)):
                        nc.tensor.matmul(
                            ps[:], w_sb[:, k, 0:128], xt[:, k, sl],
                            start=(k == 0), stop=(k == KD - 1),
                        )
            for n, sl, pq, pk in pro:
                nc.vector.tensor_copy(qT_sb[:, 0, sl], pq[:])
                nc.vector.tensor_copy(kT_sb[:, 0, sl], pk[:])
                est_pe[0] += 2 * 4096 * 0.417
            for st in range(min(TPB, ST)):
                emit_unit(("v", st))

            # Block order: pair-0 runs j0,j1 before pair-1 starts, and the
            # ACT-heaviest blocks (j3) sit mid-schedule.  The pair-lag means
            # pair-1's projection prereqs flow through the filler pacing of
            # the late ACT-heavy blocks instead of being consumed early.
            border = [(g, j) for j in j_order for g in range(NPAIR)]
            if NJ >= 4 and NPAIR > 1:
                # pair-lag: pair-1's j>=2 projection prereqs then flow
                # through the filler pacing of the ACT-heavy late blocks
                border = ([(0, 0), (0, 1), (1, 0), (1, 1)]
                          + [(0, j) for j in j_order[2:]]
                          + [(1, j) for j in j_order[2:]])
            last_block_j = border[-1][1]

            # filler supply, in rough consumption order (arrival-gated)
            for st in range(TPB, min(2 * TPB, ST)):
                filler_q.append(("v", st))
            for n in range(min(2, NJ)):
                if MQ > 1:
                    filler_q.append(("q", 1, n))
                    filler_q.append(("k", 1, n))
            for st in range(2 * TPB, ST):
                filler_q.append(("v", st))
            for n in range(2, NJ):
                filler_q.append(("q", 0, n))
                filler_q.append(("k", 0, n))
            for n in range(2, NJ):
                if MQ > 1:
                    filler_q.append(("q", 1, n))
                    filler_q.append(("k", 1, n))

            for g, j in border:
                # hard prerequisites for this block
                emit_unit(("q", g, j))
                for c in range(j + 1):
                    emit_unit(("k", g, c))
                attention_block(g, j)

            # tail: flush deferred norms and remaining filler units
            while deferred:
                tick_deferred()
                for u in o_defer:
                    filler_q.append(u)
                o_defer.clear()
                flush_fillers()
            for u in o_defer:
                filler_q.append(u)
            o_defer.clear()
            flush_fillers()

    nc.compile()
    return nc


def _get_program():
    key = (S, D, HL, DH)
    if key not in _PROGRAM_CACHE:
        _PROGRAM_CACHE[key] = build_program(*key)
    return _PROGRAM_CACHE[key]


def make_in_maps(x, Wq, Wk, Wv, Wo):
    import ml_dtypes

    bf = ml_dtypes.bfloat16
    x = np.asarray(x, dtype=np.float32)
    Wq = np.asarray(Wq, dtype=np.float32)
    Wk = np.asarray(Wk, dtype=np.float32)
    Wv = np.asarray(Wv, dtype=np.float32)
    Wo = np.asarray(Wo, dtype=np.float32)
    xTs = [np.ascontiguousarray(x[b].T).astype(bf) for b in range(B)]
    in_maps = []
    for c in range(NCORES):
        b, g = divmod(c, NCORES // B)
        sl = slice(HL * DH * g, HL * DH * (g + 1))
        in_maps.append(
            {
                "xT": xTs[b],
                "wq": np.ascontiguousarray(Wq[sl, :].T).astype(bf),
                "wk": np.ascontiguousarray(Wk[sl, :].T).astype(bf),
                "wv": np.ascontiguousarray(Wv[sl, :].T).astype(bf),
                "wo": np.ascontiguousarray(Wo[:, sl].T).astype(bf),
            }
        )
    return in_maps


def kernel(x, Wq, Wk, Wv, Wo, bo):
    from concourse import bass2jax

    nc = _get_program()
    in_maps = make_in_maps(x, Wq, Wk, Wv, Wo)
    res = bass2jax.run_bass_via_pjrt(nc, in_maps, n_cores=NCORES)
    outs = [np.asarray(res[c]["out"], dtype=np.float32) for c in range(NCORES)]
    gpb = NCORES // B
    o = np.stack([sum(outs[b * gpb + g] for g in range(gpb)) for b in range(B)])
    o = o + np.asarray(bo, dtype=np.float32)[None, None, :]
    return o.astype(np.float32)
